# revision 1
# baseline (speedup 1.0000x reference)
"""FFT-based DCT-II on 8 trn2 NeuronCores (pipelined rev E).

Per core (256 rows): Makhoul DCT->real-FFT, four-step radix-64x64, twiddles
folded into stage-2 tables, conjugate symmetry (66 stage-1 slots incl. two
zero columns), mid-transpose via DRAM roundtrip. fp16 operands, fp32 psum.

Pipelining: x1 loaded in 4 chunks; stage-1 copybacks stream into 4 t_sb
tiles whose T-writes overlap stage 1; T2 read in m-chunks overlapping
stage 2; DMAs spread over sync/scalar/gpsimd queues.
"""

import numpy as np

N = 4096
R = 2048
RPC = 256

_state = {}


def _tables():
    n1 = np.arange(64)[:, None].astype(np.float64)
    j = np.arange(33)[None, :].astype(np.float64)
    F1c = np.cos(2 * np.pi * n1 * j / 64)
    F1s = -np.sin(2 * np.pi * n1 * j / 64)
    F1 = np.concatenate([F1c, F1s], axis=1)  # [64, 66]; cols 33 & 65 are 0
    f1_np = np.vstack([F1, F1]).astype(np.float16)  # [128, 66]

    n2v = np.arange(64)[:, None].astype(np.float64)
    k2v = np.arange(64)[None, :].astype(np.float64)

    def HH_single(k1):
        k = 64 * k2v + k1
        Gc = np.cos(2 * np.pi * n2v * k / N)
        Gs = -np.sin(2 * np.pi * n2v * k / N)
        cosE = np.cos(np.pi * k / (2 * N))
        sinE = np.sin(np.pi * k / (2 * N))
        sigma = 1.0 if k1 <= 32 else -1.0
        H1 = cosE * Gc + sinE * Gs
        H2 = sigma * (sinE * Gc - cosE * Gs)
        return np.concatenate([H1, H2], axis=0)  # [128, 64]

    HH = np.zeros((33, 128, 128))
    for a in range(1, 32):
        HH[a][:, :64] = HH_single(a)
        HH[a][:, 64:] = HH_single(64 - a)
    HH[0][:, :64] = HH_single(0)
    HH[32][:, 64:] = HH_single(32)
    # t2 partitions come from the (n c) DMA merge: p = 2*n2 + c
    rowperm = np.empty(128, dtype=np.int64)
    for n2 in range(64):
        for c in range(2):
            rowperm[2 * n2 + c] = c * 64 + n2
    HH = HH[:, rowperm, :]
    hh_np = HH.transpose(1, 0, 2).astype(np.float16).copy()  # [128, 33, 128]

    k1_arr = np.empty(64, dtype=np.int64)
    for a in range(32):
        k1_arr[2 * a] = a
        k1_arr[2 * a + 1] = (64 - a) if a > 0 else 32
    return f1_np, hh_np, k1_arr


def _t2_slice(t2_tiles, a):
    if a < 8:
        return t2_tiles[0][:, a, :]
    if a == 32:
        return t2_tiles[0][:, 8, :]
    j = 1 + (a - 8) // 8
    return t2_tiles[j][:, (a - 8) % 8, :]


def _build():
    import concourse.tile as tile
    from concourse import bacc, mybir

    f16 = mybir.dt.float16
    f32 = mybir.dt.float32

    nc = bacc.Bacc("TRN2", target_bir_lowering=False, debug=False, num_devices=8)
    x1_d = nc.dram_tensor("x1", [128, 8192], f16, kind="ExternalInput").ap()
    f1_d = nc.dram_tensor("f1", [128, 66], f16, kind="ExternalInput").ap()
    hh_d = nc.dram_tensor("hh", [128, 33, 128], f16, kind="ExternalInput").ap()
    y_d = nc.dram_tensor("y", [32, 2, 64, 256], f32, kind="ExternalOutput").ap()

    with tile.TileContext(nc) as tc:
        with (
            tc.tile_pool(name="const", bufs=1) as const,
            tc.tile_pool(name="data", bufs=1) as data,
            tc.tile_pool(name="dram", bufs=1, space="DRAM") as dram,
            tc.tile_pool(name="ps1", bufs=3, space="PSUM") as ps1,
            tc.tile_pool(name="ps2", bufs=2, space="PSUM") as ps2,
            tc.tile_pool(name="ysb", bufs=6) as ysb,
        ):
            f1_sb = const.tile([128, 66], f16)
            hh_sb = const.tile([128, 33, 128], f16)
            nc.sync.dma_start(f1_sb[:], f1_d)

            # x1 in 4 chunks (sync queue); hh deferred behind them
            x1_g = []
            for g in range(4):
                xg = data.tile([128, 2048], f16, name=f"x1_{g}")
                nc.sync.dma_start(xg[:], x1_d[:, 2048 * g : 2048 * g + 2048])
                x1_g.append(xg)
            nc.sync.dma_start(hh_sb[:], hh_d)

            # T in DRAM slot-major [s=(c,m), n2, r]: contiguous fast writes;
            # reads split by c across scalar/gpsimd queues (disjoint
            # partition halves -> disjoint DMA-engine sets, parallel).
            t_dram = dram.tile([64, 2, 33, 256], f16)  # [n2, c, m, r]
            t_sb_g = [
                data.tile([66, 16, 256], f16, name=f"tsb_{g}") for g in range(4)
            ]

            # stage 1: f in [0,16), psum tile per (p=f//2, h) holds 2 MMs.
            # Emit h-alternating so adjacent PE matmuls hit different row
            # groups and overlap in the array.
            cb = 0
            for p in range(8):
                tiles = [
                    ps1.tile([66, 2, 512], f32, name=f"s1ps_{p}_{h}", tag="s1ps")
                    for h in range(2)
                ]
                for j in range(2):
                    for h in range(2):
                        f = 2 * p + j
                        g, sl = f // 4, (f % 4) * 512
                        nc.tensor.matmul(
                            tiles[h][:, j, :],
                            f1_sb[64 * h : 64 * h + 64, :],
                            x1_g[g][64 * h : 64 * h + 64, sl : sl + 512],
                            start=True,
                            stop=True,
                        )
                for h in range(2):
                    dst = t_sb_g[p // 2][
                        :, (p % 2) * 8 : (p % 2) * 8 + 8, 128 * h : 128 * h + 128
                    ]
                    src = tiles[h][:].rearrange("s j (a b) -> s (j a) b", a=4)
                    if cb % 2 == 0:
                        nc.vector.tensor_copy(dst, src)
                    else:
                        nc.scalar.copy(dst, src)
                    cb += 1
                # write this n2 8-slice as soon as both h halves are done
                nc.sync.dma_start(
                    t_dram[8 * p : 8 * p + 8].rearrange("n c m r -> (c m) n r"),
                    t_sb_g[p // 2][:, (p % 2) * 8 : (p % 2) * 8 + 8, :],
                )

            # T2 read in m-chunks; per chunk: c=0 half on scalar queue,
            # c=1 half on gpsimd queue (parallel). Chunk0 carries m=32 too.
            t2_tiles = [
                data.tile([128, 9 if j == 0 else 8, 256], f16, name=f"t2_{j}")
                for j in range(4)
            ]

            # full-width reads via the (n c) partition merge
            t_rd = t_dram[:].rearrange("n c m r -> (n c) m r")
            nc.scalar.dma_start(t2_tiles[0][:, 0:8, :], t_rd[:, 0:8, :])
            nc.scalar.dma_start(t2_tiles[0][:, 8:9, :], t_rd[:, 32:33, :])
            for j in range(1, 4):
                nc.scalar.dma_start(
                    t2_tiles[j][:, 0:8, :], t_rd[:, 8 * j : 8 * j + 8, :]
                )

            # stage 2: 16 psum tiles, each two a's; a=0 accumulates m=0 and m=32
            for q in range(16):
                ps = ps2.tile([128, 512], f32)
                for i in range(2):
                    a = 2 * q + i
                    out = ps[:, 256 * i : 256 * i + 256]
                    if a == 0:
                        nc.tensor.matmul(
                            out, hh_sb[:, 0, :], _t2_slice(t2_tiles, 0),
                            start=True, stop=False,
                        )
                        nc.tensor.matmul(
                            out, hh_sb[:, 32, :], _t2_slice(t2_tiles, 32),
                            start=False, stop=True,
                        )
                    else:
                        nc.tensor.matmul(
                            out, hh_sb[:, a, :], _t2_slice(t2_tiles, a),
                            start=True, stop=True,
                        )
                y_sb = ysb.tile([128, 512], f32)
                if q % 2 == 0:
                    nc.vector.tensor_copy(y_sb[:], ps[:])
                else:
                    nc.scalar.copy(y_sb[:], ps[:])
                dst = y_d[2 * q : 2 * q + 2].rearrange("a d k r -> (d k) a r")
                src = y_sb[:].rearrange("p (a r) -> p a r", a=2)
                if q % 2 == 0:
                    nc.sync.dma_start(dst, src)
                else:
                    nc.scalar.dma_start(dst, src)

    nc.compile()
    return nc


def _pack_x1(x_rows):
    v = np.empty_like(x_rows)
    v[:, : N // 2] = x_rows[:, 0::2]
    v[:, N // 2 :] = x_rows[:, 1::2][:, ::-1]
    x1 = v.reshape(2, 128, 64, 64).transpose(0, 2, 3, 1).reshape(128, 8192)
    return np.ascontiguousarray(x1.astype(np.float16))


def kernel(x, _trace: bool = False):
    from concourse.bass_utils import run_bass_kernel_spmd

    x = np.asarray(x, dtype=np.float32)
    assert x.shape == (R, N)
    if "nc" not in _state:
        _state["nc"] = _build()
        _state["tables"] = _tables()
    nc = _state["nc"]
    f1_np, hh_np, k1_arr = _state["tables"]

    in_maps = []
    for c in range(8):
        in_maps.append(
            {
                "x1": _pack_x1(x[c * RPC : (c + 1) * RPC]),
                "f1": f1_np,
                "hh": hh_np,
            }
        )

    res = run_bass_kernel_spmd(nc, in_maps, list(range(8)), trace=_trace)

    y = np.empty((R, N), dtype=np.float32)
    for c in range(8):
        ydev = res.results[c]["y"]  # [32, 2, 64, 256]
        perm = ydev.transpose(3, 2, 0, 1).reshape(RPC, 64, 64)
        yc = np.empty((RPC, 64, 64), dtype=np.float32)
        yc[:, :, k1_arr] = perm
        y[c * RPC : (c + 1) * RPC] = yc.reshape(RPC, N)
    if _trace:
        _state["last_result"] = res
    return y

